# revision 10
# baseline (speedup 1.0000x reference)
"""GNN message-passing kernel for Trainium2 (8 NeuronCores, SPMD).

Strategy (edge-parallel by destination):
  * Host renumbers nodes into 128-node windows via degree-sorted greedy
    bin-packing so every window carries ~1276 edges; windows are dealt to
    (core, slot) pairs so per-slot edge counts match across cores (one
    shared compile-time tile schedule for all 8 cores, ~1.2% edge pad).
  * Host pre-gathers x[row] / edge_attr per edge (transposed), folds the
    per-edge scale wrc = wts / max(count[col], 1) INTO the eax rows
    (w*relu(h) == relu(w*h) for w>=0, h linear in eax), and ships eax as
    fp8-e4m3 — halving the dominant HBM stream.  The scatter one-hot S is
    then exact 0/1.
  * eax groups stream via SWDGE (gpsimd) so descriptors spread over all
    16 SDMA engines; colof / xu load once up front.
  * Device, per 128-edge tile: h = eaxT.T @ mw1_aug (PSUM, AGRP tiles
    share a PSUM region + one ACT relu -> G fp8), S one-hots generated
    SGRP tiles at a time with one DVE tensor_tensor is_equal, then
    scatter-accumulate T_w[hid, node] += G.T @ S into a per-window column
    of a shared PSUM bank (4 windows per bank).
  * Per 4-window group: one bf16 cast of T4, then update MLP at N=512:
    h2 = M2R.T @ T4 + uw1axb.T @ xu  (mw2 pre-folded into uw1's recv rows
    on the host: M2R = mw2 @ uw1[64:128]), relu, out = uw2.T @ h2r + ub2,
    written transposed in bf16.
  * Host inverts the node permutation and returns [N, 64] float32.
"""
import heapq

import numpy as np
import ml_dtypes

import concourse.bacc as bacc
import concourse.tile as tile
from concourse import mybir
from concourse.bass_utils import run_bass_kernel_spmd

BF = mybir.dt.bfloat16
F32 = mybir.dt.float32
F8 = mybir.dt.float8e4
bf16 = ml_dtypes.bfloat16
f8e4 = ml_dtypes.float8_e4m3fn

P = 128
NCORES = 8
HID = 128
NODE_D = 64
EDGE_D = 32
GLOB_D = 32
FEAT = NODE_D + EDGE_D + 1   # 97: x | edge_attr | wrc (ones folded w/ wrc)
XU_P = NODE_D + GLOB_D + 2   # 98: x | u | s | ones

# const blob column layout (bf16, [128, BLOB_W])
_B_MW1 = 0      # [0:97, 0:128]     mw1_aug (bf16 rhs for mm1)
_B_M2R = 128    # [0:128, 128:256]  mw2 @ uw1[64:128]  (recv path folded)
_B_UAXB = 256   # [0:98, 256:384]   uw1[0:64] | uw1[128:160] | v | ub1
_B_UW2 = 384    # [0:128, 384:448]  uw2
_B_IOTA = 448   # [0:128, 448:448+SGRPMAX*128] iota row 0..127 repeated
SGRPMAX = 8
BLOB_W = _B_IOTA + SGRPMAX * P

# tunables (defaults used by the grading path)
CFG = {
    "group": 128,         # 128-edge tiles per eax DMA transfer (need not
                          # divide nt; tail transfer is smaller)
    "agrp": 5,            # tiles sharing one PSUM h4 region + one relu
    "sgrp": 8,            # S one-hot tiles generated per DVE op (0 = per-tile)
    "wgrp": 4,            # windows per update-MLP batch (PSUM bank share)
    "geax": 2, "ework": 3, "swork": 3, "nwork": 3, "gout": 2,
    "ph": 2, "pt": 2,
    "g_dtype": "bf16",    # relu output (scatter lhsT) dtype: f8 | bf16
    "relu_act": 3,        # of every 5 relu chunks, this many go to ACT
    "relu_mod": 5,
    "hh_engine": "act",   # dve | act
    "h2r_engine": "dve",  # dve | act
    "outb_engine": "act",  # dve | act
}

_program_cache: dict = {}
_last_results = None  # BassKernelResults of the most recent run (for profiling)


def _build_program(t_sched):
    """Build + finalize the SPMD Bass program for a tile schedule."""
    GROUP = CFG["group"]
    AGRP = CFG["agrp"]
    SGRP = CFG["sgrp"]
    WGRP = CFG["wgrp"]
    GD = F8 if CFG["g_dtype"] == "f8" else BF
    nt = sum(t_sched)
    e_pad = nt * P
    nslots = len(t_sched)
    nsh = nslots * P
    assert SGRP == SGRPMAX  # staircase iota blob layout is built for SGRPMAX

    nc = bacc.Bacc()
    eax_d = nc.dram_tensor("eax", [FEAT, e_pad], F8, kind="ExternalInput")
    colof_d = nc.dram_tensor("colof", [P, nt], BF, kind="ExternalInput")
    blob_d = nc.dram_tensor("blob", [P, BLOB_W], BF, kind="ExternalInput")
    ub2_d = nc.dram_tensor("ub2", [64, 1], F32, kind="ExternalInput")
    xu_d = nc.dram_tensor("xu", [XU_P, nsh], BF, kind="ExternalInput")
    out_d = nc.dram_tensor("out", [64, nsh], BF, kind="ExternalOutput")

    with tile.TileContext(nc) as tc:
        with (
            tc.tile_pool(name="consts", bufs=1) as consts,
            tc.tile_pool(name="geax", bufs=CFG["geax"]) as geax,
            tc.tile_pool(name="ework", bufs=CFG["ework"]) as ework,
            tc.tile_pool(name="swork", bufs=CFG["swork"]) as swork,
            tc.tile_pool(name="nwork", bufs=CFG["nwork"]) as nwork,
            tc.tile_pool(name="gout", bufs=CFG["gout"]) as gout,
            tc.tile_pool(name="ph", bufs=CFG["ph"], space="PSUM") as ph,
            tc.tile_pool(name="pt", bufs=CFG["pt"], space="PSUM") as pt,
            tc.tile_pool(name="p2", bufs=1, space="PSUM") as p2,
            tc.tile_pool(name="po", bufs=1, space="PSUM") as po,
        ):
            blob_t = consts.tile([P, BLOB_W], BF)
            nc.sync.dma_start(blob_t[:], blob_d[:])
            mw1_t = blob_t[0:FEAT, _B_MW1:_B_MW1 + HID]
            m2r_t = blob_t[0:HID, _B_M2R:_B_M2R + HID]
            uaxb_t = blob_t[0:XU_P, _B_UAXB:_B_UAXB + HID]
            uw2_t = blob_t[0:HID, _B_UW2:_B_UW2 + 64]
            iota_t = blob_t[:, _B_IOTA:_B_IOTA + SGRPMAX * P]
            colof_t = consts.tile([P, nt], BF)
            nc.sync.dma_start(colof_t[:], colof_d[:])
            ub2_t = consts.tile([64, 1], F32)
            nc.gpsimd.dma_start(ub2_t[:], ub2_d[:])
            xu_t = consts.tile([XU_P, nsh], BF)
            nc.gpsimd.dma_start(xu_t[:], xu_d[:])

            def emit_update(jg, nw, t4_ps):
                # batched update MLP over nw windows (N = nw*128)
                ncol = nw * P
                hh = nwork.tile([HID, WGRP * P], BF, tag="Hh")
                if CFG["hh_engine"] == "act":
                    nc.scalar.copy(hh[:, 0:ncol], t4_ps[:, 0:ncol])
                else:
                    nc.vector.tensor_copy(hh[:, 0:ncol], t4_ps[:, 0:ncol])
                h2_ps = p2.tile([HID, WGRP * P], F32, space="PSUM")
                nc.tensor.matmul(h2_ps[:, 0:ncol], lhsT=m2r_t,
                                 rhs=hh[:, 0:ncol], start=True, stop=False)
                nc.tensor.matmul(h2_ps[:, 0:ncol], lhsT=uaxb_t,
                                 rhs=xu_t[:, jg * P:jg * P + ncol],
                                 start=False, stop=True)
                h2r = nwork.tile([HID, WGRP * P], BF, tag="h2r")
                if CFG["h2r_engine"] == "act":
                    nc.scalar.activation(
                        h2r[:, 0:ncol], h2_ps[:, 0:ncol],
                        mybir.ActivationFunctionType.Relu,
                    )
                else:
                    nc.vector.tensor_scalar_max(
                        h2r[:, 0:ncol], h2_ps[:, 0:ncol], 0.0)
                o_ps = po.tile([64, WGRP * P], F32, space="PSUM")
                nc.tensor.matmul(o_ps[:, 0:ncol], lhsT=uw2_t,
                                 rhs=h2r[:, 0:ncol], start=True, stop=True)
                o_g = gout.tile([64, WGRP * P], BF, tag="o")
                if CFG["outb_engine"] == "act":
                    nc.scalar.activation(
                        o_g[:, 0:ncol], o_ps[:, 0:ncol],
                        mybir.ActivationFunctionType.Identity,
                        bias=ub2_t[:, 0:1],
                    )
                else:
                    nc.vector.tensor_scalar(
                        out=o_g[:, 0:ncol], in0=o_ps[:, 0:ncol],
                        scalar1=ub2_t[:, 0:1], scalar2=None,
                        op0=mybir.AluOpType.add,
                    )
                nc.gpsimd.dma_start(
                    out_d[:, jg * P:jg * P + ncol], o_g[:, 0:ncol],
                )

            eax_g = None
            s_g = None
            t = 0
            nchunk = 0
            nsgen = 0
            pending = []        # deferred scatter MMs (one chunk behind)
            prev_update = None  # deferred update-MLP args (one group behind)
            for jg in range(0, nslots, WGRP):
                nw = min(WGRP, nslots - jg)
                t4_ps = pt.tile([HID, WGRP * P], F32, space="PSUM")
                for jr in range(nw):
                    tj = t_sched[jg + jr]
                    kdone = 0
                    while kdone < tj:
                        nk = min(AGRP, tj - kdone)
                        h4_ps = ph.tile([P, AGRP * HID], F32, space="PSUM")
                        chunk = []
                        for i in range(nk):
                            g, r = divmod(t, GROUP)
                            if r == 0:
                                n = min(GROUP, nt - g * GROUP)
                                eax_g = geax.tile([FEAT, GROUP * P], F8,
                                                  tag="eax")
                                nc.gpsimd.dma_start(
                                    eax_g[:, 0:n * P],
                                    eax_d[:, g * GROUP * P:
                                          (g * GROUP + n) * P],
                                )
                            sg, sr = divmod(t, SGRP)
                            if sr == 0:
                                sn = min(SGRP, nt - sg * SGRP)
                                # S in [p, col, tile] layout: all operands
                                # innermost-contiguous -> DVE 2x packed mode
                                s_g = swork.tile([P, P, SGRP], BF, tag="S")
                                nc.vector.tensor_tensor(
                                    out=s_g[:, :, 0:sn],
                                    in0=iota_t.rearrange(
                                        "p (c k) -> p c k", k=SGRP)[:, :, 0:sn],
                                    in1=colof_t[:, sg * SGRP:sg * SGRP + sn]
                                    .unsqueeze(1).broadcast_to([P, P, sn]),
                                    op=mybir.AluOpType.is_equal,
                                )
                                nsgen += 1
                            s_t = s_g[:, :, sr:sr + 1]
                            nc.tensor.matmul(
                                h4_ps[:, i * HID:(i + 1) * HID],
                                lhsT=eax_g[:, r * P:(r + 1) * P], rhs=mw1_t,
                                start=True, stop=True,
                            )
                            chunk.append((i, s_t))
                            t += 1
                        g4_t = ework.tile([P, AGRP * HID], GD, tag="G")
                        if nchunk % CFG["relu_mod"] < CFG["relu_act"]:
                            nc.scalar.activation(
                                g4_t[:, 0:nk * HID], h4_ps[:, 0:nk * HID],
                                mybir.ActivationFunctionType.Relu,
                            )
                        else:
                            nc.vector.tensor_scalar_max(
                                g4_t[:, 0:nk * HID], h4_ps[:, 0:nk * HID], 0.0)
                        nchunk += 1
                        # emit the PREVIOUS chunk's scatters now — keeps the
                        # tensor queue from blocking on this chunk's relu
                        for mm in pending:
                            nc.tensor.matmul(**mm)
                        pending = [
                            dict(out=t4_ps[:, jr * P:(jr + 1) * P],
                                 lhsT=g4_t[:, i * HID:(i + 1) * HID],
                                 rhs=s_t,
                                 start=(kdone + i == 0),
                                 stop=(kdone + i == tj - 1))
                            for i, s_t in chunk
                        ]
                        kdone += nk
                        if prev_update is not None:
                            # previous group's scatters are all emitted by
                            # now — safe to emit its update MLP
                            emit_update(*prev_update)
                            prev_update = None
                prev_update = (jg, nw, t4_ps)
            for mm in pending:
                nc.tensor.matmul(**mm)
            emit_update(*prev_update)
    nc.finalize()
    return nc


def _pack_nodes(deg, n_nodes):
    """Greedy balanced bin-packing of nodes into 128-node windows so each
    window's edge total is ~equal.  Returns win/offset per node and
    per-window edge counts."""
    nblk = -(-n_nodes // P)
    nslots = -(-nblk // NCORES)
    nwin = nslots * NCORES

    order = np.argsort(-deg, kind="stable")
    win_of = np.empty(n_nodes, np.int32)
    off_of = np.empty(n_nodes, np.int32)
    wcount = np.zeros(nwin, np.int32)
    wedges = np.zeros(nwin, np.int64)
    heap = [(0, w) for w in range(nwin)]
    heapq.heapify(heap)
    for nd in order:
        while True:
            esum, w = heapq.heappop(heap)
            if wcount[w] < P:
                break
        win_of[nd] = w
        off_of[nd] = wcount[w]
        wcount[w] += 1
        d = int(deg[nd])
        wedges[w] = esum + d
        if wcount[w] < P:
            heapq.heappush(heap, (esum + d, w))
    return win_of, off_of, wedges, nslots


def kernel(x, edge_index, edge_attr, u, node_batch, wts,
           mw1, mb1, mw2, mb2, uw1, ub1, uw2, ub2):
    x = np.asarray(x, np.float32)
    edge_index = np.asarray(edge_index)
    edge_attr = np.asarray(edge_attr, np.float32)
    u = np.asarray(u, np.float32)
    node_batch = np.asarray(node_batch).astype(np.int64)
    wts = np.asarray(wts, np.float32).reshape(-1)
    mw1 = np.asarray(mw1, np.float32)
    mb1 = np.asarray(mb1, np.float32)
    mw2 = np.asarray(mw2, np.float32)
    mb2 = np.asarray(mb2, np.float32)
    uw1 = np.asarray(uw1, np.float32)
    ub1 = np.asarray(ub1, np.float32)
    uw2 = np.asarray(uw2, np.float32)
    ub2 = np.asarray(ub2, np.float32)

    GROUP = CFG["group"]
    n_nodes = x.shape[0]
    row = np.asarray(edge_index[0], np.int64)
    col = np.asarray(edge_index[1], np.int64)

    # per-node stats (host): count, 1/max(cnt,1), weight-sum
    cnt = np.bincount(col, minlength=n_nodes)
    rc = 1.0 / np.maximum(cnt, 1.0).astype(np.float32)
    wsum = np.bincount(col, weights=wts, minlength=n_nodes).astype(np.float32)
    s_node = wsum * rc

    # node -> (window, offset) packing, windows -> (slot, core)
    win_of, off_of, wedges, nslots = _pack_nodes(cnt, n_nodes)
    nwin = nslots * NCORES
    nsh = nslots * P
    rank = np.argsort(-wedges, kind="stable")
    win_assign = rank.reshape(nslots, NCORES)             # [slot, core]
    grp_max = wedges[win_assign].max(axis=1)
    t_sched = np.maximum(1, -(-grp_max // P)).astype(np.int64)
    pad = (-int(t_sched.sum())) % GROUP
    t_sched[-1] += pad
    t_sched = [int(v) for v in t_sched]
    nt = sum(t_sched)
    e_pad = nt * P

    # per-edge
    ewin = win_of[col]
    colof = off_of[col].astype(np.float32)
    wrc = wts * rc[col]
    eorder = np.argsort(ewin, kind="stable")
    wstart = np.zeros(nwin + 1, np.int64)
    np.cumsum(np.bincount(ewin, minlength=nwin), out=wstart[1:])

    # nodes grouped by window
    norder = np.argsort(win_of, kind="stable")
    nstart = np.zeros(nwin + 1, np.int64)
    np.cumsum(np.bincount(win_of, minlength=nwin), out=nstart[1:])

    key = tuple(t_sched)
    if key not in _program_cache:
        _program_cache[key] = _build_program(t_sched)
    nc = _program_cache[key]

    # const blob (shared by all cores)
    v_row = mb2 @ uw1[NODE_D:2 * NODE_D, :]              # [HID]
    blob = np.zeros((P, BLOB_W), np.float32)
    blob[0:NODE_D + EDGE_D, _B_MW1:_B_MW1 + HID] = mw1
    blob[NODE_D + EDGE_D, _B_MW1:_B_MW1 + HID] = mb1
    blob[0:HID, _B_M2R:_B_M2R + HID] = mw2 @ uw1[NODE_D:2 * NODE_D, :]
    blob[0:NODE_D, _B_UAXB:_B_UAXB + HID] = uw1[0:NODE_D, :]
    blob[NODE_D:NODE_D + GLOB_D, _B_UAXB:_B_UAXB + HID] = uw1[2 * NODE_D:, :]
    blob[NODE_D + GLOB_D, _B_UAXB:_B_UAXB + HID] = v_row
    blob[NODE_D + GLOB_D + 1, _B_UAXB:_B_UAXB + HID] = ub1
    blob[0:HID, _B_UW2:_B_UW2 + 64] = uw2
    # staircase iota: column (c*SGRP + k) holds value c  (S is generated in
    # [p, col, tile] layout so every DVE operand is innermost-contiguous)
    blob[:, _B_IOTA:_B_IOTA + SGRPMAX * P] = np.repeat(
        np.arange(P, dtype=np.float32), SGRPMAX)[None, :]
    blob_bf = blob.astype(bf16)
    ub2_a = ub2.reshape(64, 1).astype(np.float32)

    u_per_node = u[node_batch]                           # [N, GLOB_D]

    # slot offsets within a core's edge stream
    slot_off = np.zeros(nslots + 1, np.int64)
    np.cumsum(np.asarray(t_sched) * P, out=slot_off[1:])

    in_maps = []
    node_idx_cores = []
    for c in range(NCORES):
        eidx = np.full(e_pad, -1, np.int64)
        nidx = np.full(nsh, -1, np.int64)
        for j in range(nslots):
            w = int(win_assign[j, c])
            m = int(wstart[w + 1] - wstart[w])
            o = slot_off[j]
            eidx[o:o + m] = eorder[wstart[w]:wstart[w] + m]
            nl = norder[nstart[w]:nstart[w + 1]]
            nidx[j * P + off_of[nl]] = nl
        evalid = eidx >= 0
        eidxc = np.where(evalid, eidx, 0)
        # eax: wrc * [x[row] | edge_attr | 1] transposed, zeros on pads
        wrce = wrc[eidxc].astype(np.float32)
        wrce[~evalid] = 0.0
        eax = np.empty((e_pad, FEAT), np.float32)
        eax[:, 0:NODE_D] = x[row[eidxc]]
        eax[:, NODE_D:NODE_D + EDGE_D] = edge_attr[eidxc]
        eax[:, FEAT - 1] = 1.0
        eax *= wrce[:, None]
        np.clip(eax, -240.0, 240.0, out=eax)
        cvec = np.full(e_pad, 255.0, np.float32)
        cvec[evalid] = colof[eidx[evalid]]
        # p-major colof: colof_pm[p, t] = col-offset of edge (t*128+p)
        colof_pm = np.ascontiguousarray(
            cvec.reshape(nt, P).T).astype(bf16)

        nvalid = nidx >= 0
        nidxc = np.where(nvalid, nidx, 0)
        xu = np.zeros((nsh, XU_P), np.float32)
        xu[:, 0:NODE_D] = x[nidxc]
        xu[:, NODE_D:NODE_D + GLOB_D] = u_per_node[nidxc]
        xu[:, NODE_D + GLOB_D] = s_node[nidxc]
        xu[:, NODE_D + GLOB_D + 1] = 1.0
        xu[~nvalid] = 0.0

        in_maps.append({
            "eax": np.ascontiguousarray(eax.T).astype(f8e4),
            "colof": colof_pm,
            "blob": blob_bf,
            "ub2": ub2_a,
            "xu": np.ascontiguousarray(xu.T).astype(bf16),
        })
        node_idx_cores.append((nidx, nvalid))

    res = run_bass_kernel_spmd(nc, in_maps, core_ids=list(range(NCORES)))
    global _last_results
    _last_results = res

    out_full = np.zeros((n_nodes, 64), np.float32)
    for c in range(NCORES):
        nidx, nvalid = node_idx_cores[c]
        oc = np.asarray(res.results[c]["out"], dtype=np.float32)  # [64, nsh]
        out_full[nidx[nvalid]] = oc.T[nvalid]
    return out_full


# revision 11
# speedup vs baseline: 2.3133x; 2.3133x over previous
"""GNN message-passing kernel for Trainium2 (8 NeuronCores, SPMD).

Strategy (edge-parallel by destination):
  * Host renumbers nodes into 128-node windows via degree-sorted greedy
    bin-packing so every window carries ~1276 edges; windows are dealt to
    (core, slot) pairs so per-slot edge counts match across cores (one
    shared compile-time tile schedule for all 8 cores, ~1.2% edge pad).
  * Host pre-gathers x[row] / edge_attr per edge (transposed), folds the
    per-edge scale wrc = wts / max(count[col], 1) INTO the eax rows
    (w*relu(h) == relu(w*h) for w>=0, h linear in eax), and ships eax as
    fp8-e4m3 — halving the dominant HBM stream.  The scatter one-hot S is
    then exact 0/1.
  * eax groups stream via SWDGE (gpsimd) so descriptors spread over all
    16 SDMA engines; colof / xu load once up front.
  * Device, per 128-edge tile: h = eaxT.T @ mw1_aug (PSUM, AGRP tiles
    share a PSUM region + one ACT relu -> G fp8), S one-hots generated
    SGRP tiles at a time with one DVE tensor_tensor is_equal, then
    scatter-accumulate T_w[hid, node] += G.T @ S into a per-window column
    of a shared PSUM bank (4 windows per bank).
  * Per 4-window group: one bf16 cast of T4, then update MLP at N=512:
    h2 = M2R.T @ T4 + uw1axb.T @ xu  (mw2 pre-folded into uw1's recv rows
    on the host: M2R = mw2 @ uw1[64:128]), relu, out = uw2.T @ h2r + ub2,
    written transposed in bf16.
  * Host inverts the node permutation and returns [N, 64] float32.
"""
import heapq

import numpy as np
import ml_dtypes

import concourse.bacc as bacc
import concourse.tile as tile
from concourse import mybir
from concourse.bass_utils import run_bass_kernel_spmd

BF = mybir.dt.bfloat16
F32 = mybir.dt.float32
F8 = mybir.dt.float8e4
bf16 = ml_dtypes.bfloat16
f8e4 = ml_dtypes.float8_e4m3fn

P = 128
NCORES = 8
HID = 128
NODE_D = 64
EDGE_D = 32
GLOB_D = 32
FEAT = NODE_D + EDGE_D + 1   # 97: x | edge_attr | wrc (ones folded w/ wrc)
XU_P = NODE_D + GLOB_D + 2   # 98: x | u | s | ones

# const blob column layout (bf16, [128, BLOB_W])
_B_MW1 = 0      # [0:97, 0:128]     mw1_aug (bf16 rhs for mm1)
_B_M2R = 128    # [0:128, 128:256]  mw2 @ uw1[64:128]  (recv path folded)
_B_UAXB = 256   # [0:98, 256:384]   uw1[0:64] | uw1[128:160] | v | ub1
_B_UW2 = 384    # [0:128, 384:448]  uw2
_B_IOTA = 448   # [0:128, 448:448+SGRPMAX*128] iota row 0..127 repeated
SGRPMAX = 8
_B_ONE8 = _B_IOTA + SGRPMAX * P   # [0:128, +8] ones (local_scatter data)
BLOB_W = _B_ONE8 + SGRPMAX

# tunables (defaults used by the grading path)
CFG = {
    "group": 32,          # 128-edge tiles per eax DMA transfer
    "agrp": 5,            # tiles sharing one PSUM h4 region + one relu
    "sgrp": 8,            # S one-hot tiles generated per batched op
    "wgrp": 4,            # windows per update-MLP batch (PSUM bank share)
    "geax": 6, "ework": 3, "swork": 3, "nwork": 3, "gout": 2,
    "ph": 2, "pt": 2,
    "g_dtype": "bf16",    # relu output (scatter lhsT) dtype: f8 | bf16
    "relu_act": 3,        # of every 5 relu chunks, this many go to ACT
    "relu_mod": 5,
    "s_dve": 2,           # of every s_mod S batches, this many on DVE
    "s_mod": 4,           # (rest on GPSIMD local_scatter)
    "hh_engine": "act",   # dve | act
    "h2r_engine": "dve",  # dve | act
    "outb_engine": "act",  # dve | act
}

_program_cache: dict = {}
_last_results = None  # BassKernelResults of the most recent run (for profiling)


def _build_program(t_sched):
    """Build + finalize the SPMD Bass program for a tile schedule."""
    GROUP = CFG["group"]
    AGRP = CFG["agrp"]
    SGRP = CFG["sgrp"]
    WGRP = CFG["wgrp"]
    GD = F8 if CFG["g_dtype"] == "f8" else BF
    nt = sum(t_sched)
    e_pad = nt * P
    nslots = len(t_sched)
    nsh = nslots * P
    assert SGRP == SGRPMAX  # staircase iota blob layout is built for SGRPMAX

    nc = bacc.Bacc()
    eax_d = nc.dram_tensor("eax", [FEAT, e_pad], F8, kind="ExternalInput")
    colof_d = nc.dram_tensor("colof", [P, nt], BF, kind="ExternalInput")
    colofi_d = nc.dram_tensor("colofi", [P, nt], mybir.dt.int16,
                              kind="ExternalInput")
    blob_d = nc.dram_tensor("blob", [P, BLOB_W], BF, kind="ExternalInput")
    ub2_d = nc.dram_tensor("ub2", [64, 1], F32, kind="ExternalInput")
    xu_d = nc.dram_tensor("xu", [XU_P, nsh], BF, kind="ExternalInput")
    out_d = nc.dram_tensor("out", [64, nsh], BF, kind="ExternalOutput")

    with tile.TileContext(nc) as tc:
        with (
            tc.tile_pool(name="consts", bufs=1) as consts,
            tc.tile_pool(name="geax", bufs=CFG["geax"]) as geax,
            tc.tile_pool(name="ework", bufs=CFG["ework"]) as ework,
            tc.tile_pool(name="swork", bufs=CFG["swork"]) as swork,
            tc.tile_pool(name="nwork", bufs=CFG["nwork"]) as nwork,
            tc.tile_pool(name="gout", bufs=CFG["gout"]) as gout,
            tc.tile_pool(name="ph", bufs=CFG["ph"], space="PSUM") as ph,
            tc.tile_pool(name="pt", bufs=CFG["pt"], space="PSUM") as pt,
            tc.tile_pool(name="p2", bufs=1, space="PSUM") as p2,
            tc.tile_pool(name="po", bufs=1, space="PSUM") as po,
        ):
            blob_t = consts.tile([P, BLOB_W], BF)
            nc.sync.dma_start(blob_t[:], blob_d[:])
            mw1_t = blob_t[0:FEAT, _B_MW1:_B_MW1 + HID]
            m2r_t = blob_t[0:HID, _B_M2R:_B_M2R + HID]
            uaxb_t = blob_t[0:XU_P, _B_UAXB:_B_UAXB + HID]
            uw2_t = blob_t[0:HID, _B_UW2:_B_UW2 + 64]
            iota_t = blob_t[:, _B_IOTA:_B_IOTA + SGRPMAX * P]
            colof_t = consts.tile([P, nt], BF)
            nc.sync.dma_start(colof_t[:], colof_d[:])
            colofi_t = consts.tile([P, nt], mybir.dt.int16)
            nc.sync.dma_start(colofi_t[:], colofi_d[:])
            one8_t = blob_t[:, _B_ONE8:_B_ONE8 + SGRPMAX]
            ub2_t = consts.tile([64, 1], F32)
            nc.gpsimd.dma_start(ub2_t[:], ub2_d[:])
            xu_t = consts.tile([XU_P, nsh], BF)
            nc.gpsimd.dma_start(xu_t[:], xu_d[:])

            def emit_update(jg, nw, t4_ps):
                # batched update MLP over nw windows (N = nw*128)
                ncol = nw * P
                hh = nwork.tile([HID, WGRP * P], BF, tag="Hh")
                if CFG["hh_engine"] == "act":
                    nc.scalar.copy(hh[:, 0:ncol], t4_ps[:, 0:ncol])
                else:
                    nc.vector.tensor_copy(hh[:, 0:ncol], t4_ps[:, 0:ncol])
                h2_ps = p2.tile([HID, WGRP * P], F32, space="PSUM")
                nc.tensor.matmul(h2_ps[:, 0:ncol], lhsT=m2r_t,
                                 rhs=hh[:, 0:ncol], start=True, stop=False)
                nc.tensor.matmul(h2_ps[:, 0:ncol], lhsT=uaxb_t,
                                 rhs=xu_t[:, jg * P:jg * P + ncol],
                                 start=False, stop=True)
                h2r = nwork.tile([HID, WGRP * P], BF, tag="h2r")
                if CFG["h2r_engine"] == "act":
                    nc.scalar.activation(
                        h2r[:, 0:ncol], h2_ps[:, 0:ncol],
                        mybir.ActivationFunctionType.Relu,
                    )
                else:
                    nc.vector.tensor_scalar_max(
                        h2r[:, 0:ncol], h2_ps[:, 0:ncol], 0.0)
                o_ps = po.tile([64, WGRP * P], F32, space="PSUM")
                nc.tensor.matmul(o_ps[:, 0:ncol], lhsT=uw2_t,
                                 rhs=h2r[:, 0:ncol], start=True, stop=True)
                o_g = gout.tile([64, WGRP * P], BF, tag="o")
                if CFG["outb_engine"] == "act":
                    nc.scalar.activation(
                        o_g[:, 0:ncol], o_ps[:, 0:ncol],
                        mybir.ActivationFunctionType.Identity,
                        bias=ub2_t[:, 0:1],
                    )
                else:
                    nc.vector.tensor_scalar(
                        out=o_g[:, 0:ncol], in0=o_ps[:, 0:ncol],
                        scalar1=ub2_t[:, 0:1], scalar2=None,
                        op0=mybir.AluOpType.add,
                    )
                nc.gpsimd.dma_start(
                    out_d[:, jg * P:jg * P + ncol], o_g[:, 0:ncol],
                )

            eax_g = None
            s_g = None
            t = 0
            nchunk = 0
            nsgen = 0
            pending = []        # deferred scatter MMs (one chunk behind)
            prev_update = None  # deferred update-MLP args (one group behind)
            for jg in range(0, nslots, WGRP):
                nw = min(WGRP, nslots - jg)
                t4_ps = pt.tile([HID, WGRP * P], F32, space="PSUM")
                for jr in range(nw):
                    tj = t_sched[jg + jr]
                    kdone = 0
                    while kdone < tj:
                        nk = min(AGRP, tj - kdone)
                        h4_ps = ph.tile([P, AGRP * HID], F32, space="PSUM")
                        chunk = []
                        for i in range(nk):
                            g, r = divmod(t, GROUP)
                            if r == 0:
                                n = min(GROUP, nt - g * GROUP)
                                eax_g = geax.tile([FEAT, GROUP * P], F8,
                                                  tag="eax")
                                nc.gpsimd.dma_start(
                                    eax_g[:, 0:n * P],
                                    eax_d[:, g * GROUP * P:
                                          (g * GROUP + n) * P],
                                )
                            sg, sr = divmod(t, SGRP)
                            if sr == 0:
                                sn = min(SGRP, nt - sg * SGRP)
                                s_g = swork.tile([P, SGRP * P], BF, tag="S")
                                use_gp = (nsgen % CFG["s_mod"]
                                          >= CFG["s_dve"]) and sn % 2 == 0
                                if use_gp:
                                    nc.gpsimd.local_scatter(
                                        s_g[:, 0:sn * P],
                                        one8_t[:, 0:sn],
                                        colofi_t[:, sg * SGRP:sg * SGRP + sn],
                                        channels=P, num_elems=sn * P,
                                        num_idxs=sn,
                                    )
                                else:
                                    nc.vector.tensor_tensor(
                                        out=s_g[:, 0:sn * P],
                                        in0=iota_t[:, 0:sn * P],
                                        in1=colof_t[:, sg * SGRP:sg * SGRP + sn]
                                        .unsqueeze(2).broadcast_to([P, sn, P]),
                                        op=mybir.AluOpType.is_equal,
                                    )
                                nsgen += 1
                            s_t = s_g[:, sr * P:(sr + 1) * P]
                            nc.tensor.matmul(
                                h4_ps[:, i * HID:(i + 1) * HID],
                                lhsT=eax_g[:, r * P:(r + 1) * P], rhs=mw1_t,
                                start=True, stop=True,
                            )
                            chunk.append((i, s_t))
                            t += 1
                        g4_t = ework.tile([P, AGRP * HID], GD, tag="G")
                        if nchunk % CFG["relu_mod"] < CFG["relu_act"]:
                            nc.scalar.activation(
                                g4_t[:, 0:nk * HID], h4_ps[:, 0:nk * HID],
                                mybir.ActivationFunctionType.Relu,
                            )
                        else:
                            nc.vector.tensor_scalar_max(
                                g4_t[:, 0:nk * HID], h4_ps[:, 0:nk * HID], 0.0)
                        nchunk += 1
                        # emit the PREVIOUS chunk's scatters now — keeps the
                        # tensor queue from blocking on this chunk's relu
                        for mm in pending:
                            nc.tensor.matmul(**mm)
                        pending = [
                            dict(out=t4_ps[:, jr * P:(jr + 1) * P],
                                 lhsT=g4_t[:, i * HID:(i + 1) * HID],
                                 rhs=s_t,
                                 start=(kdone + i == 0),
                                 stop=(kdone + i == tj - 1))
                            for i, s_t in chunk
                        ]
                        kdone += nk
                        if prev_update is not None:
                            # previous group's scatters are all emitted by
                            # now — safe to emit its update MLP
                            emit_update(*prev_update)
                            prev_update = None
                prev_update = (jg, nw, t4_ps)
            for mm in pending:
                nc.tensor.matmul(**mm)
            emit_update(*prev_update)
    nc.finalize()
    return nc


def _pack_nodes(deg, n_nodes):
    """Greedy balanced bin-packing of nodes into 128-node windows so each
    window's edge total is ~equal.  Returns win/offset per node and
    per-window edge counts."""
    nblk = -(-n_nodes // P)
    nslots = -(-nblk // NCORES)
    nwin = nslots * NCORES

    order = np.argsort(-deg, kind="stable")
    win_of = np.empty(n_nodes, np.int32)
    off_of = np.empty(n_nodes, np.int32)
    wcount = np.zeros(nwin, np.int32)
    wedges = np.zeros(nwin, np.int64)
    heap = [(0, w) for w in range(nwin)]
    heapq.heapify(heap)
    for nd in order:
        while True:
            esum, w = heapq.heappop(heap)
            if wcount[w] < P:
                break
        win_of[nd] = w
        off_of[nd] = wcount[w]
        wcount[w] += 1
        d = int(deg[nd])
        wedges[w] = esum + d
        if wcount[w] < P:
            heapq.heappush(heap, (esum + d, w))
    return win_of, off_of, wedges, nslots


def kernel(x, edge_index, edge_attr, u, node_batch, wts,
           mw1, mb1, mw2, mb2, uw1, ub1, uw2, ub2):
    x = np.asarray(x, np.float32)
    edge_index = np.asarray(edge_index)
    edge_attr = np.asarray(edge_attr, np.float32)
    u = np.asarray(u, np.float32)
    node_batch = np.asarray(node_batch).astype(np.int64)
    wts = np.asarray(wts, np.float32).reshape(-1)
    mw1 = np.asarray(mw1, np.float32)
    mb1 = np.asarray(mb1, np.float32)
    mw2 = np.asarray(mw2, np.float32)
    mb2 = np.asarray(mb2, np.float32)
    uw1 = np.asarray(uw1, np.float32)
    ub1 = np.asarray(ub1, np.float32)
    uw2 = np.asarray(uw2, np.float32)
    ub2 = np.asarray(ub2, np.float32)

    GROUP = CFG["group"]
    n_nodes = x.shape[0]
    row = np.asarray(edge_index[0], np.int64)
    col = np.asarray(edge_index[1], np.int64)

    # per-node stats (host): count, 1/max(cnt,1), weight-sum
    cnt = np.bincount(col, minlength=n_nodes)
    rc = 1.0 / np.maximum(cnt, 1.0).astype(np.float32)
    wsum = np.bincount(col, weights=wts, minlength=n_nodes).astype(np.float32)
    s_node = wsum * rc

    # node -> (window, offset) packing, windows -> (slot, core)
    win_of, off_of, wedges, nslots = _pack_nodes(cnt, n_nodes)
    nwin = nslots * NCORES
    nsh = nslots * P
    rank = np.argsort(-wedges, kind="stable")
    win_assign = rank.reshape(nslots, NCORES)             # [slot, core]
    grp_max = wedges[win_assign].max(axis=1)
    t_sched = np.maximum(1, -(-grp_max // P)).astype(np.int64)
    pad = (-int(t_sched.sum())) % GROUP
    t_sched[-1] += pad
    t_sched = [int(v) for v in t_sched]
    nt = sum(t_sched)
    e_pad = nt * P

    # per-edge
    ewin = win_of[col]
    colof = off_of[col].astype(np.float32)
    wrc = wts * rc[col]
    eorder = np.argsort(ewin, kind="stable")
    wstart = np.zeros(nwin + 1, np.int64)
    np.cumsum(np.bincount(ewin, minlength=nwin), out=wstart[1:])

    # nodes grouped by window
    norder = np.argsort(win_of, kind="stable")
    nstart = np.zeros(nwin + 1, np.int64)
    np.cumsum(np.bincount(win_of, minlength=nwin), out=nstart[1:])

    key = tuple(t_sched)
    if key not in _program_cache:
        _program_cache[key] = _build_program(t_sched)
    nc = _program_cache[key]

    # const blob (shared by all cores)
    v_row = mb2 @ uw1[NODE_D:2 * NODE_D, :]              # [HID]
    blob = np.zeros((P, BLOB_W), np.float32)
    blob[0:NODE_D + EDGE_D, _B_MW1:_B_MW1 + HID] = mw1
    blob[NODE_D + EDGE_D, _B_MW1:_B_MW1 + HID] = mb1
    blob[0:HID, _B_M2R:_B_M2R + HID] = mw2 @ uw1[NODE_D:2 * NODE_D, :]
    blob[0:NODE_D, _B_UAXB:_B_UAXB + HID] = uw1[0:NODE_D, :]
    blob[NODE_D:NODE_D + GLOB_D, _B_UAXB:_B_UAXB + HID] = uw1[2 * NODE_D:, :]
    blob[NODE_D + GLOB_D, _B_UAXB:_B_UAXB + HID] = v_row
    blob[NODE_D + GLOB_D + 1, _B_UAXB:_B_UAXB + HID] = ub1
    blob[0:HID, _B_UW2:_B_UW2 + 64] = uw2
    blob[:, _B_IOTA:_B_IOTA + SGRPMAX * P] = np.tile(
        np.arange(P, dtype=np.float32)[None, :], (1, SGRPMAX))
    blob[:, _B_ONE8:_B_ONE8 + SGRPMAX] = 1.0
    blob_bf = blob.astype(bf16)
    ub2_a = ub2.reshape(64, 1).astype(np.float32)

    u_per_node = u[node_batch]                           # [N, GLOB_D]

    # slot offsets within a core's edge stream
    slot_off = np.zeros(nslots + 1, np.int64)
    np.cumsum(np.asarray(t_sched) * P, out=slot_off[1:])

    in_maps = []
    node_idx_cores = []
    for c in range(NCORES):
        eidx = np.full(e_pad, -1, np.int64)
        nidx = np.full(nsh, -1, np.int64)
        for j in range(nslots):
            w = int(win_assign[j, c])
            m = int(wstart[w + 1] - wstart[w])
            o = slot_off[j]
            eidx[o:o + m] = eorder[wstart[w]:wstart[w] + m]
            nl = norder[nstart[w]:nstart[w + 1]]
            nidx[j * P + off_of[nl]] = nl
        evalid = eidx >= 0
        eidxc = np.where(evalid, eidx, 0)
        # eax: wrc * [x[row] | edge_attr | 1] transposed, zeros on pads
        wrce = wrc[eidxc].astype(np.float32)
        wrce[~evalid] = 0.0
        eax = np.empty((e_pad, FEAT), np.float32)
        eax[:, 0:NODE_D] = x[row[eidxc]]
        eax[:, NODE_D:NODE_D + EDGE_D] = edge_attr[eidxc]
        eax[:, FEAT - 1] = 1.0
        eax *= wrce[:, None]
        np.clip(eax, -240.0, 240.0, out=eax)
        cvec = np.full(e_pad, 255.0, np.float32)
        cvec[evalid] = colof[eidx[evalid]]
        # p-major colof: colof_pm[p, t] = col-offset of edge (t*128+p)
        colof_pm = np.ascontiguousarray(
            cvec.reshape(nt, P).T).astype(bf16)
        # int16 variant for gpsimd local_scatter: global column within the
        # SGRP-tile row (t%SGRP)*128 + colof, -1 on pads (ignored)
        civec = np.full(e_pad, -1, np.int64)
        civec[evalid] = colof[eidx[evalid]].astype(np.int64)
        karr = (np.arange(nt, dtype=np.int64) % SGRPMAX) * P
        ci = civec.reshape(nt, P)
        ci = np.where(ci >= 0, ci + karr[:, None], -1)
        colof_i16 = np.ascontiguousarray(ci.T).astype(np.int16)

        nvalid = nidx >= 0
        nidxc = np.where(nvalid, nidx, 0)
        xu = np.zeros((nsh, XU_P), np.float32)
        xu[:, 0:NODE_D] = x[nidxc]
        xu[:, NODE_D:NODE_D + GLOB_D] = u_per_node[nidxc]
        xu[:, NODE_D + GLOB_D] = s_node[nidxc]
        xu[:, NODE_D + GLOB_D + 1] = 1.0
        xu[~nvalid] = 0.0

        in_maps.append({
            "eax": np.ascontiguousarray(eax.T).astype(f8e4),
            "colof": colof_pm,
            "colofi": colof_i16,
            "blob": blob_bf,
            "ub2": ub2_a,
            "xu": np.ascontiguousarray(xu.T).astype(bf16),
        })
        node_idx_cores.append((nidx, nvalid))

    res = run_bass_kernel_spmd(nc, in_maps, core_ids=list(range(NCORES)))
    global _last_results
    _last_results = res

    out_full = np.zeros((n_nodes, 64), np.float32)
    for c in range(NCORES):
        nidx, nvalid = node_idx_cores[c]
        oc = np.asarray(res.results[c]["out"], dtype=np.float32)  # [64, nsh]
        out_full[nidx[nvalid]] = oc.T[nvalid]
    return out_full
